# revision 11
# baseline (speedup 1.0000x reference)
"""CNN + truncated path-signature (depth 4) + FF head on 8 TRN2 NeuronCores.

Strategy
--------
- Batch data-parallel signature computation: core c handles batches
  [8c, 8c+8) = 32 (batch, out_ch) lanes, T=128 time steps on partitions.
- Signature reformulated to avoid sequential scans:
    dx, P1 (= shifted path), Y (= suffix sums) are free;
    the only prefix scan (level 2, S2pre) is one triangular matmul L @ M2;
    S3 = B^T X;  S4 = B^T R + tmp81'^T (dx(x)dx)/2   (suffix-vector trick),
  so levels 3 and 4 are plain T-contractions on the TensorEngine.
- All prep tensors (dx, ut, u2, at, dxh, yt) are [T, T] linear maps of the
  path p, computed as TensorE matmuls against constant matrices (PE is idle
  during prep; this removes the shift-DMA and DRAM broadcast bounce from the
  critical path).
- The 60 MB w0 is row-sharded 8 ways (the memory roofline win): AllToAll
  re-shards the signature activations BATCH-MAJOR ([64, 3712] rows = global
  batches), each core multiplies its [3712, 512] w0 shard, ReduceScatter
  returns each core its own 8 batches, and the small w1/w2 head finishes
  per-core.  The batch-major exchange makes both the pack and the post-A2A
  reload fat: one XBAR transpose-DMA turns zex [64, 3712] into the K-major
  [128, 29*64] lhsT tile directly.
- w0 rows are permuted host-side to match the kernel's natural feature
  order (and zero-padded 3690 -> 3712 per shard), so no on-device
  transposes are needed.
"""
import os
import sys
sys.path.insert(0, "/opt/trn_rl_repo")
if os.environ.get("JAX_PLATFORMS") == "cpu":
    # The SPMD launch needs the axon/neuron PJRT backend.
    os.environ["JAX_PLATFORMS"] = ""

import numpy as np
import bass_rust as _bass_rust
import concourse.bass as bass
import concourse.tile as tile
import concourse.mybir as mybir
from concourse.vector_clock import ScopedClock
from concourse.bass_utils import run_bass_kernel_spmd

F32 = mybir.dt.float32
F32R = mybir.dt.float32r
BF16 = mybir.dt.bfloat16
AL = mybir.AluOpType
AF = mybir.ActivationFunctionType

NCORES = 8
B, T, IN_CH = 64, 128, 32
OUT_CH, CH, D = 4, 8, 9            # conv out-channels, conv width, path dim
BL = B // NCORES                   # local batches = 8
LANES = BL * OUT_CH                # 32 lanes/core
NG = 4                             # lane groups of 8
GL = 8                             # lanes per group
SIGC = 7380                        # per-lane signature channels
KSH = OUT_CH * SIGC // NCORES      # w0 K-shard rows per core = 3690
KSHP = 29 * 128                    # padded shard rows = 3712
H0, H1, NCLS = 512, 256, 10


class _SplitDrainTileContext(tile.TileContext):
    """Tile exit drain carries one sem wait per CTRL instruction.

    This container's walrus build rejects >2 sync waits on a CTRL
    instruction; Tile's exit drain waits on the whole global clock.
    Redistribute the waits over nops on the same engine (program order on
    one engine preserves semantics)."""

    MAX_WAITS = 1

    def _split_body_waits(self):
        """Move excess sem waits from any instruction onto preceding nops on
        the same engine (same-engine program order preserves semantics)."""
        nc = self.nc
        for bb in nc.main_func.blocks:
            heavy = [ins for ins in bb.instructions
                     if ins.sync_info and ins.sync_info.on_wait
                     and len(ins.sync_info.on_wait) > self.MAX_WAITS]
            if not heavy:
                continue
            extra = {}
            for ins in heavy:
                w = list(ins.sync_info.on_wait)
                ins.sync_info.on_wait = w[:self.MAX_WAITS]
                nops = []
                for i in range(self.MAX_WAITS, len(w), self.MAX_WAITS):
                    n = nc.engines[ins.engine].nop(hint="wait_split")
                    # pop the freshly appended nop from wherever it landed
                    for bb2 in nc.main_func.blocks:
                        if bb2.instructions and bb2.instructions[-1] is n.ins:
                            bb2.instructions.pop()
                            break
                    for wt in w[i:i + self.MAX_WAITS]:
                        handle = _bass_rust.SemaphoreHandle(wt.ant_name, wt.id)
                        _bass_rust.wait_op(n.ins, handle, wt.wait_value,
                                           "sem-ge", False)
                    nops.append(n.ins)
                extra[id(ins)] = nops
            new_list = []
            for ins in bb.instructions:
                new_list.extend(extra.get(id(ins), ()))
                new_list.append(ins)
            bb.instructions[:] = new_list

    def _drain_and_barrier(self, tick_clock, wait_clock):
        nc = self.nc
        self._split_body_waits()
        probe = nc.sync.nop(hint="tile_exit_wait_0")
        wait_clock.add_sem_waits(
            probe.ins, ScopedClock({None: tick_clock.global_clock})
        )
        waits = list(probe.ins.sync_info.on_wait or [])
        probe.ins.sync_info.on_wait = waits[:1]
        for w in waits[1:]:
            n = nc.sync.nop(hint="tile_exit_wait")
            handle = _bass_rust.SemaphoreHandle(w.ant_name, w.id)
            _bass_rust.wait_op(n.ins, handle, w.wait_value, "sem-ge", False)
        nc.sync.drain()
        nc.all_engine_barrier()
        assert self.sems is not None
        popped = nc._tile_sem_poison_stack.pop()
        assert popped is self._sem_poison
        nc.clear_and_free_semaphores(list(self.sems.allocated().values()))
        nc.all_engine_barrier()


def _ap(t, extra, *dims):
    """AP over tile t's buffer: partition dim from the tile, custom free dims.

    dims[0] may override the partition [step, count]."""
    base = t[:]
    return bass.AP(base.tensor, base.offset + extra, list(dims))


def _w0_perm():
    """Row permutation p s.t. w0_permuted[i] = w0[p[i]] matches the kernel's
    feature order: per oc: [S1(9) | S2(81)] then rows 90 + ij*90 + c with
    c<81 -> level4 (ij,kl=c), c>=81 -> level3 (ij, k=c-81)."""
    p = np.empty(OUT_CH * SIGC, np.int64)
    i = 0
    for oc in range(OUT_CH):
        base = oc * SIGC
        p[i:i + 90] = base + np.arange(90)          # S1 then S2, native order
        i += 90
        for ij in range(81):
            # level-4 block (ij, kl) for kl in 0..80 -> orig 819 + ij*81 + kl
            p[i:i + 81] = base + 819 + ij * 81 + np.arange(81)
            i += 81
            # level-3 (ij, k) for k in 0..8 -> orig 90 + ij*9 + k
            p[i:i + 9] = base + 90 + ij * 9 + np.arange(9)
            i += 9
    assert i == OUT_CH * SIGC
    return p


def _prep_mats():
    """Six [T, T] lhsT matrices M with out[m] = sum_t M[t, m] p[t]:
    dx, at (= P1 + dx/2), ut (= P1/2 + dx/6), u2 (= P1/3 + dx/12),
    dxh (= dx/2), yt (= p[T-1] - p)."""
    I = np.eye(T, dtype=np.float32)
    E1 = np.eye(T, k=1, dtype=np.float32)       # E1[t, m] = d(m, t+1)
    m_dx = I - E1
    m_at = 0.5 * (E1 + I)
    m_ut = E1 / 3.0 + I / 6.0
    m_u2 = E1 / 4.0 + I / 12.0
    m_dxh = 0.5 * (I - E1)
    m_yt = -I.copy()
    m_yt[T - 1, :] += 1.0
    return np.concatenate([m_dx, m_at, m_ut, m_u2, m_dxh, m_yt],
                          axis=1)  # [128, 768]


def _build():
    nc = bass.Bass(num_devices=NCORES, target_bir_lowering=True, trn_type="TRN2")

    # ---- per-core DRAM inputs ----
    xs = nc.dram_tensor("xs", [T, BL, IN_CH], F32, kind="ExternalInput")
    cwr = nc.dram_tensor("cwr", [128, 16], F32, kind="ExternalInput")
    cbr = nc.dram_tensor("cbr", [128, OUT_CH], F32, kind="ExternalInput")
    tlin = nc.dram_tensor("tlin", [128, 1], F32, kind="ExternalInput")
    ltri = nc.dram_tensor("ltri", [128, 128], F32R, kind="ExternalInput")
    pmats = nc.dram_tensor("pmats", [128, 6 * 128], F32R, kind="ExternalInput")
    onef = nc.dram_tensor("onef", [128, 1], F32, kind="ExternalInput")
    oner = nc.dram_tensor("oner", [128, 2], F32R, kind="ExternalInput")
    idn = nc.dram_tensor("idn", [128, 128], F32, kind="ExternalInput")
    w0s = nc.dram_tensor("w0s", [KSHP, H0], BF16, kind="ExternalInput")
    w1s = nc.dram_tensor("w1s", [H0, H1], F32, kind="ExternalInput")
    w2s = nc.dram_tensor("w2s", [H1, NCLS], F32, kind="ExternalInput")
    b0c = nc.dram_tensor("b0c", [H0, 1], F32, kind="ExternalInput")
    b1c = nc.dram_tensor("b1c", [H1, 1], F32, kind="ExternalInput")
    b2r = nc.dram_tensor("b2r", [BL, NCLS], F32, kind="ExternalInput")
    out = nc.dram_tensor("out", [BL, NCLS], F32, kind="ExternalOutput")

    with _SplitDrainTileContext(nc) as tc:
        with tc.tile_pool(name="dram", bufs=1, space="DRAM") as dram:
            zl2 = dram.tile([B, KSHP], BF16)      # batch-major z, my batches
            zex2 = dram.tile([B, KSHP], BF16)     # post-A2A: all batches, my K
            cin = dram.tile([B, H0], F32)
            cout = dram.tile([BL, H0], F32)
            prow = dram.tile([1, LANES * D], F32)  # p[T-1] bounce (S1 only)

            with tc.tile_pool(name="const", bufs=1) as cpool, \
                 tc.tile_pool(name="prep", bufs=1) as ppool:
                # ---- scalar queue: critical-path consts ----
                xs_sb = cpool.tile([128, BL * IN_CH], F32)
                nc.scalar.dma_start(xs_sb[:], _ap(xs, 0, [BL * IN_CH, 128],
                                                [1, BL * IN_CH]))
                cw_sb = cpool.tile([128, 16], F32)
                nc.scalar.dma_start(cw_sb[:], cwr[:])
                cb_sb = cpool.tile([128, OUT_CH], F32)
                nc.scalar.dma_start(cb_sb[:], cbr[:])
                tl_sb = cpool.tile([128, 1], F32)
                nc.scalar.dma_start(tl_sb[:], tlin[:])
                pm_sb = cpool.tile([128, 6 * 128], F32R)
                nc.scalar.dma_start(pm_sb[:], pmats[:])
                lt_sb = cpool.tile([128, 128], F32R)
                nc.scalar.dma_start(lt_sb[:], ltri[:])
                oner_sb = cpool.tile([128, 2], F32R)
                nc.scalar.dma_start(oner_sb[:], oner[:])
                onef_sb = cpool.tile([128, 1], F32)
                nc.scalar.dma_start(onef_sb[:], onef[:])
                # ---- sync queue: the big w0 shard as one streaming DMA ----
                w0all = cpool.tile([128, 29 * H0], BF16, tag="w0all",
                                   name="w0all")
                nc.sync.dma_start(
                    _ap(w0all, 0, [29 * H0, 128], [H0, 29], [1, H0]),
                    _ap(w0s, 0, [H0, 128], [128 * H0, 29], [1, H0]))
                # ---- gpsimd queue: non-critical consts ----
                zpad = cpool.tile([B, KSHP - KSH], BF16, tag="zpad")
                nc.gpsimd.memset(zpad[:], 0.0)
                nc.gpsimd.dma_start(
                    _ap(zl2, KSH, [KSHP, B], [1, KSHP - KSH]), zpad[:])
                idn_sb = cpool.tile([128, 128], F32)
                nc.gpsimd.dma_start(idn_sb[:], idn[:])
                w1_sb = [cpool.tile([128, H1], F32, tag=f"w1_{j}", name=f"w1_{j}")
                         for j in range(4)]
                for j in range(4):
                    nc.gpsimd.dma_start(w1_sb[j][:], w1s[j * 128:(j + 1) * 128, :])
                w2_sb = [cpool.tile([128, NCLS], F32, tag=f"w2_{j}", name=f"w2_{j}")
                         for j in range(2)]
                for j in range(2):
                    nc.gpsimd.dma_start(w2_sb[j][:], w2s[j * 128:(j + 1) * 128, :])
                b0_sb = cpool.tile([128, 4], F32)
                nc.gpsimd.dma_start(b0_sb[:], _ap(b0c, 0, [1, 128], [128, 4]))
                b1_sb = cpool.tile([128, 2], F32)
                nc.gpsimd.dma_start(b1_sb[:], _ap(b1c, 0, [1, 128], [128, 2]))
                b2_sb = cpool.tile([BL, NCLS], F32)
                nc.gpsimd.dma_start(b2_sb[:], b2r[:])

                # ---- prep: conv -> path p, then the six T-linear maps ----
                W = LANES * D  # 288
                p = ppool.tile([128, W], F32, tag="p")
                pr = ppool.tile([128, W], F32R, tag="pr")
                dx = ppool.tile([128, W], F32, tag="dx")
                at = ppool.tile([128, W], F32, tag="at")
                ut = ppool.tile([128, W], F32, tag="ut")
                u2 = ppool.tile([128, W], F32, tag="u2")
                dxh = ppool.tile([128, W], F32, tag="dxh")
                yt = ppool.tile([128, W], F32, tag="yt")

                # conv, batched over (oc, b, ch): p_data = sum_k x_k * w[oc,k]
                pdst = _ap(p, 1, [W, 128], [D, OUT_CH], [4 * D, BL], [1, CH])
                tmpc = ppool.tile([128, BL * IN_CH], F32, tag="tmpc")
                tview = _ap(tmpc, 0, [BL * IN_CH, 128],
                            [CH * BL, OUT_CH], [CH, BL], [1, CH])
                for k in range(4):
                    xsv = _ap(xs_sb, k, [BL * IN_CH, 128],
                              [0, OUT_CH], [IN_CH, BL], [4, CH])
                    cwv = _ap(cw_sb, k, [16, 128], [4, OUT_CH], [0, BL], [0, CH])
                    if k == 0:
                        nc.vector.tensor_tensor(pdst, xsv, cwv, AL.mult)
                    else:
                        nc.vector.tensor_tensor(tview, xsv, cwv, AL.mult)
                        nc.vector.tensor_tensor(pdst, pdst, tview, AL.add)
                cbv = _ap(cb_sb, 0, [OUT_CH, 128], [1, OUT_CH], [0, BL], [0, CH])
                nc.vector.tensor_tensor(pdst, pdst, cbv, AL.add)
                # time channel into col 0 of every lane
                nc.vector.tensor_copy(_ap(p, 0, [W, 128], [D, LANES]),
                                      _ap(tl_sb, 0, [1, 128], [0, LANES]))
                # p[T-1] row bounce (only for S1; off the critical path)
                nc.gpsimd.dma_start(prow[:], p[127:128, :])
                z1tmp = ppool.tile([9, LANES], F32, tag="z1tmp")
                nc.gpsimd.dma_start(
                    z1tmp[:], _ap(prow, 0, [1, 9], [D, LANES]))
                z1all = cpool.tile([9, LANES], BF16, tag="z1all", name="z1all")
                nc.scalar.activation(z1all[:], z1tmp[:], AF.Copy)

                # six prep tensors via PE: out = M^T @ p
                nc.vector.tensor_copy(pr[:], p[:])
                prep_specs = [(dx, 0), (at, 1), (ut, 2), (u2, 3),
                              (dxh, 4), (yt, 5)]
                with tc.tile_pool(name="ppsum", bufs=1, space="PSUM") as ppsum:
                    pp = [ppsum.tile([128, W], F32, tag=f"pp{i}",
                                     name=f"pp{i}")
                          for i in range(6)]
                    for i, (dst, blk) in enumerate(prep_specs):
                        nc.tensor.matmul(
                            pp[i][:], _ap(pm_sb, blk * 128, [6 * 128, 128],
                                          [1, 128]),
                            pr[:], start=True, stop=True)
                        eng = nc.vector if i % 2 == 0 else nc.scalar
                        if eng is nc.vector:
                            eng.tensor_copy(dst[:], pp[i][:])
                        else:
                            eng.activation(dst[:], pp[i][:], AF.Copy)

                    # preload ACT function tables off the critical tail
                    dum = ppool.tile([1, 4], F32, tag="dum")
                    nc.scalar.activation(dum[0:1, 0:1], tl_sb[0:1, 0:1],
                                         AF.Sigmoid)
                    nc.scalar.activation(dum[0:1, 1:2], tl_sb[0:1, 0:1], AF.Exp)
                    nc.scalar.activation(dum[0:1, 2:3], onef_sb[0:1, 0:1], AF.Ln)

                with tc.tile_pool(name="grp", bufs=2) as gpool, \
                     tc.tile_pool(name="ps2", bufs=2, space="PSUM") as ps2, \
                     tc.tile_pool(name="ptab", bufs=1, space="PSUM") as ptab, \
                     tc.tile_pool(name="ps12", bufs=1, space="PSUM") as ps12:
                    for g in range(NG):
                        off = g * GL * D  # col offset into the 288-wide tiles
                        GW = GL * 81      # 648

                        def o_ij(t, st=1):  # [lane, i(step), j(bcast)] view
                            return _ap(t, off, [W, 128], [D, GL], [st, D], [0, D])

                        def o_ji(t, st=1):  # [lane, i(bcast), j(step)] view
                            return _ap(t, off, [W, 128], [D, GL], [0, D], [st, D])

                        m2 = gpool.tile([128, GW], F32R, tag="m2")
                        m2v = _ap(m2, 0, [GW, 128], [81, GL], [D, D], [1, D])
                        nc.vector.tensor_tensor(m2v, o_ij(at), o_ji(dx), AL.mult)

                        s2 = ps2.tile([128, GW], F32, tag="s2")
                        nc.tensor.matmul(s2[:, 0:512], lt_sb[:], m2[:, 0:512],
                                         start=True, stop=True)
                        nc.tensor.matmul(s2[:, 512:GW], lt_sb[:], m2[:, 512:GW],
                                         start=True, stop=True)

                        bt = gpool.tile([128, GW], F32R, tag="bt")
                        btv = _ap(bt, 0, [GW, 128], [81, GL], [D, D], [1, D])
                        nc.vector.tensor_tensor(btv, o_ij(ut), o_ji(dx), AL.mult)
                        nc.vector.tensor_tensor(bt[:], bt[:], s2[:], AL.add)

                        t8 = gpool.tile([128, GW], F32R, tag="t8")
                        t8v = _ap(t8, 0, [GW, 128], [81, GL], [D, D], [1, D])
                        nc.vector.tensor_tensor(t8v, o_ij(u2), o_ji(dx), AL.mult)
                        nc.vector.tensor_tensor(t8[:], t8[:], s2[:], AL.add)

                        # 82-wide lane stride: f32r matmuls need even N,
                        # so the mmTB rhs is [128, 82] with a zero last column
                        q2 = gpool.tile([128, GL * 82], F32R, tag="q2")
                        q2v = _ap(q2, 0, [GL * 82, 128], [82, GL], [D, D], [1, D])
                        nc.gpsimd.tensor_tensor(q2v, o_ij(dxh), o_ji(dx), AL.mult)
                        nc.gpsimd.tensor_scalar(
                            _ap(q2, 81, [GL * 82, 128], [82, GL]),
                            _ap(dxh, 0, [LANES * D, 128], [0, GL]),
                            0.0, None, AL.mult)

                        rx = gpool.tile([128, GL * 90], F32R, tag="rx")
                        rxv = _ap(rx, 0, [GL * 90, 128], [90, GL], [D, D], [1, D])
                        nc.gpsimd.tensor_tensor(rxv, o_ij(dx), o_ji(yt), AL.mult)
                        nc.vector.tensor_copy(
                            _ap(rx, 81, [GL * 90, 128], [90, GL], [1, D]),
                            _ap(dx, off, [W, 128], [D, GL], [1, D]))

                        tab = ptab.tile([128, 1024], F32, tag="tab")
                        s12 = ps12.tile([128, 2 * GL], F32, tag="s12")
                        # per-group staging (double-buffered so next group's
                        # evacs don't WAR-stall on this group's pack DMAs)
                        zt4g = gpool.tile([81, 720], BF16, tag="zt4g")
                        z2g = gpool.tile([81, GL], BF16, tag="z2g")
                        nc.scalar.activation(
                            _ap(z2g, 0, [GL, 81], [1, 2], [2, OUT_CH]),
                            _ap(s12, 0, [2 * GL, 81], [8, 2], [2, OUT_CH]),
                            AF.Copy)
                        for l in range(GL):
                            nc.tensor.matmul(
                                _ap(tab, 128 * l, [1024, 81], [1, 90]),
                                bt[:, l * 81:(l + 1) * 81],
                                rx[:, l * 90:(l + 1) * 90],
                                start=True, stop=False)
                            nc.tensor.matmul(s12[0:81, 2 * l:2 * l + 2],
                                             m2[:, l * 81:(l + 1) * 81],
                                             oner_sb[:], start=True, stop=True)
                            nc.tensor.matmul(
                                _ap(tab, 128 * l, [1024, 81], [1, 82]),
                                t8[:, l * 81:(l + 1) * 81],
                                q2[:, l * 82:(l + 1) * 82],
                                start=False, stop=True)
                        # batched PSUM->SBUF evacuation for the whole group:
                        # lanes l = bo*4 + oc; zt4g col = oc*180 + bo*90 + c
                        nc.vector.tensor_copy(
                            _ap(zt4g, 0, [720, 81], [180, OUT_CH], [1, 90]),
                            _ap(tab, 0, [1024, 81],
                                [128, OUT_CH], [1, 90]))
                        nc.scalar.activation(
                            _ap(zt4g, 90, [720, 81], [180, OUT_CH], [1, 90]),
                            _ap(tab, 512, [1024, 81],
                                [128, OUT_CH], [1, 90]),
                            AF.Copy)

                        # ---- pack group g into batch-major zl2 rows ----
                        # zl2 row = 16*oc + 8*h + bloc, bloc = 2g + bo;
                        # cols: h=0 -> [S1|S2|ij 0..39], h=1 -> ij 40..80.
                        # One DMA per (h, oc) so the DRAM writes walk just two
                        # rows sequentially (scattered writes are ~1 GB/s).
                        for h in range(2):
                            npart = 40 if h == 0 else 41
                            dcol = 90 if h == 0 else 0
                            poff = 0 if h == 0 else 40 * 720
                            for oc in range(OUT_CH):
                                eng = nc.scalar if (h * 4 + oc) % 2 == 0 \
                                    else nc.sync
                                eng.dma_start(
                                    _ap(zl2,
                                        (16 * oc + 8 * h + 2 * g) * KSHP
                                        + dcol,
                                        [90, npart], [KSHP, 2], [1, 90]),
                                    _ap(zt4g, oc * 180 + poff,
                                        [720, npart], [90, 2], [1, 90]))
                        # S1 (9) and S2 (81) go to the h=0 rows, cols 0..90.
                        # Feature index is on partitions here, so these DMAs
                        # are 1-elem-per-descriptor; keep them tiny per bo.
                        with nc.allow_non_contiguous_dma(reason="s12 pack"):
                            for bo in range(2):
                                nc.gpsimd.dma_start(
                                    _ap(zl2, (2 * g + bo) * KSHP, [1, 9],
                                        [16 * KSHP, OUT_CH]),
                                    _ap(z1all, 8 * g + 4 * bo, [LANES, 9],
                                        [1, OUT_CH]))
                                nc.gpsimd.dma_start(
                                    _ap(zl2, (2 * g + bo) * KSHP + 9, [1, 81],
                                        [16 * KSHP, OUT_CH]),
                                    _ap(z2g, bo, [GL, 81], [2, OUT_CH]))

                nc.gpsimd.collective_compute(
                    "AllToAll", AL.bypass,
                    replica_groups=[list(range(NCORES))],
                    ins=[zl2[:].opt()], outs=[zex2[:].opt()])

                # ---- z0 = z @ w0 partial over this core's K shard ----
                with tc.tile_pool(name="pz0", bufs=1, space="PSUM") as pz0p, \
                     tc.tile_pool(name="ptail", bufs=1, space="PSUM") as ptail:
                    # one XBAR transpose-DMA: [64, 3712] -> [128, 29*64]
                    ztall = cpool.tile([128, 29 * B], BF16, tag="ztall",
                                       name="ztall")
                    nc.scalar.dma_start_transpose(
                        _ap(ztall, 0, [29 * B, 128], [B, 29], [1, B]),
                        zex2[:])
                    # re-warm Exp/Ln tables while the matmuls run
                    nc.scalar.activation(dum[0:1, 1:2], tl_sb[0:1, 0:1], AF.Exp)
                    nc.scalar.activation(dum[0:1, 2:3], onef_sb[0:1, 0:1],
                                         AF.Ln)
                    z0p = pz0p.tile([B, H0], F32, tag="z0p")
                    for t in range(29):
                        nc.tensor.matmul(
                            z0p[:],
                            _ap(ztall, t * B, [29 * B, 128], [1, B]),
                            _ap(w0all, t * H0, [29 * H0, 128], [1, H0]),
                            start=(t == 0), stop=(t == 28))
                    z0sb = cpool.tile([B, H0], F32, tag="z0sb", name="z0sb")
                    nc.vector.tensor_copy(z0sb[:], z0p[:])
                    nc.gpsimd.dma_start(cin[:], z0sb[:])
                    nc.gpsimd.collective_compute(
                        "ReduceScatter", AL.add,
                        replica_groups=[list(range(NCORES))],
                        ins=[cin[:].opt()], outs=[cout[:].opt()])

                    # ---- tail: sigmoid(w0 out) -> w1 -> sigmoid -> w2 ----
                    z1row = cpool.tile([BL, H0], F32, tag="z1row")
                    nc.gpsimd.dma_start(z1row[:], cout[:])
                    pz1 = ptail.tile([128, 4 * BL], F32, tag="pz1")
                    z1t = cpool.tile([128, 4 * BL], F32, tag="z1t")
                    for j in range(4):
                        nc.tensor.transpose(pz1[:, j * BL:(j + 1) * BL],
                                            z1row[:, j * 128:(j + 1) * 128],
                                            idn_sb[0:BL, 0:BL])
                        nc.scalar.activation(z1t[:, j * BL:(j + 1) * BL],
                                             pz1[:, j * BL:(j + 1) * BL],
                                             AF.Sigmoid, bias=b0_sb[:, j:j + 1])
                    pz2 = ptail.tile([128, 2 * BL], F32, tag="pz2")
                    z2t = cpool.tile([128, 2 * BL], F32, tag="z2t")
                    for m in range(2):
                        for kj in range(4):
                            nc.tensor.matmul(
                                pz2[:, m * BL:(m + 1) * BL],
                                w1_sb[kj][:, m * 128:(m + 1) * 128],
                                z1t[:, kj * BL:(kj + 1) * BL],
                                start=(kj == 0), stop=(kj == 3))
                        nc.scalar.activation(z2t[:, m * BL:(m + 1) * BL],
                                             pz2[:, m * BL:(m + 1) * BL],
                                             AF.Sigmoid, bias=b1_sb[:, m:m + 1])
                    pz3 = ptail.tile([BL, NCLS], F32, tag="pz3")
                    for m in range(2):
                        nc.tensor.matmul(pz3[:], z2t[:, m * BL:(m + 1) * BL],
                                         w2_sb[m][:], start=(m == 0),
                                         stop=(m == 1))
                    z3 = cpool.tile([BL, NCLS], F32, tag="z3")
                    nc.vector.tensor_tensor(z3[:], pz3[:], b2_sb[:], AL.add)
                    mx = cpool.tile([BL, 1], F32, tag="mx")
                    nc.vector.tensor_reduce(mx[:], z3[:], mybir.AxisListType.X,
                                            AL.max)
                    tm = cpool.tile([BL, NCLS], F32, tag="tm")
                    nc.vector.tensor_scalar(tm[:], z3[:], mx[:, 0:1], None,
                                            AL.subtract)
                    ex = cpool.tile([BL, NCLS], F32, tag="ex")
                    se = cpool.tile([BL, 1], F32, tag="se")
                    nc.scalar.activation(ex[:], tm[:], AF.Exp, accum_out=se[:])
                    ls = cpool.tile([BL, 1], F32, tag="ls")
                    nc.scalar.activation(ls[:], se[:], AF.Ln)
                    osb = cpool.tile([BL, NCLS], F32, tag="osb")
                    nc.vector.tensor_scalar(osb[:], tm[:], ls[:, 0:1], None,
                                            AL.subtract)
                    nc.gpsimd.dma_start(out[:], osb[:])
    return nc


_CACHE = {}


def kernel(x, conv_w, conv_b, w0, b0, w1, b1, w2, b2):
    x = np.ascontiguousarray(np.asarray(x, np.float32))
    conv_w = np.asarray(conv_w, np.float32)
    conv_b = np.asarray(conv_b, np.float32)
    w0 = np.asarray(w0, np.float32)
    w1 = np.ascontiguousarray(np.asarray(w1, np.float32))
    w2 = np.ascontiguousarray(np.asarray(w2, np.float32))
    b0 = np.asarray(b0, np.float32)
    b1 = np.asarray(b1, np.float32)
    b2 = np.asarray(b2, np.float32)

    if "nc" not in _CACHE:
        _CACHE["nc"] = _build()
        _CACHE["perm"] = _w0_perm()
    nc = _CACHE["nc"]
    bf16 = mybir.dt.np(BF16)
    w0p = np.ascontiguousarray(w0[_CACHE["perm"], :]).astype(bf16)
    # pad each 3690-row shard to 3712 rows with zeros
    w0p = w0p.reshape(NCORES, KSH, H0)
    w0pp = np.zeros((NCORES, KSHP, H0), bf16)
    w0pp[:, :KSH, :] = w0p

    shared = {
        "cwr": np.ascontiguousarray(
            np.broadcast_to(conv_w.reshape(1, 16), (128, 16))),
        "cbr": np.ascontiguousarray(
            np.broadcast_to(conv_b.reshape(1, OUT_CH), (128, OUT_CH))),
        "tlin": np.linspace(0.0, 1.0, T, dtype=np.float32).reshape(128, 1),
        "ltri": np.ascontiguousarray(
            np.triu(np.ones((128, 128), np.float32), 1)),
        "pmats": np.ascontiguousarray(_prep_mats()),
        "onef": np.ones((128, 1), np.float32),
        "oner": np.ones((128, 2), np.float32),
        "idn": np.eye(128, dtype=np.float32),
        "w1s": w1, "w2s": w2,
        "b0c": b0.reshape(H0, 1), "b1c": b1.reshape(H1, 1),
        "b2r": np.ascontiguousarray(np.broadcast_to(b2.reshape(1, NCLS),
                                                    (BL, NCLS))),
    }
    in_maps = []
    for c in range(NCORES):
        m = dict(shared)
        m["xs"] = np.ascontiguousarray(
            x[c * BL:(c + 1) * BL, 0].transpose(1, 0, 2))
        m["w0s"] = np.ascontiguousarray(w0pp[c])
        in_maps.append(m)

    _CACHE["in_maps"] = in_maps
    res = run_bass_kernel_spmd(nc, in_maps, core_ids=list(range(NCORES)))
    return np.concatenate([res.results[c]["out"] for c in range(NCORES)],
                          axis=0)
